# Initial kernel scaffold
#
"""Dense optical flow kernel for Trainium2, 8-core SPMD.

Pipeline (per core = one (sample, x-half) pair):
  frames -> gray/sobel features (row-polyphase, ry-major layout)
  -> l2-normalize f2 (rsqrt via ACT abs_reciprocal_sqrt + DVE Newton)
  -> replicated window tensor f2px -> 15x15 windowed correlation (f32)
  -> row/col max + first-argmax -> displacement grid -> separable gaussian
  smoothing (phase-decomposed H-pass on DVE, banded-matmul V-pass on PE)
  -> direction normalize -> full-res flow.

All DMAs move large contiguous runs (layouts chosen so src/dst runs are
>= 2KB) to keep dynamic-DGE descriptor counts low.
"""

import numpy as np

import concourse.bacc as bacc
import concourse.tile as tile
from concourse import mybir
from concourse.bass_utils import run_bass_kernel_spmd

F32 = mybir.dt.float32
Alu = mybir.AluOpType
Act = mybir.ActivationFunctionType
AX = mybir.AxisListType

H = 512
W = 512
B = 4
XL = 288          # per-core padded column span
GXL = 68          # local anchor columns (64 + 2 halo each side)
NEG = np.float32(-1.0e30)
POS = np.float32(3.0e38)


# ----------------------------------------------------------------------------
# constants (host side)
# ----------------------------------------------------------------------------

def _gaussian_sep():
    ax = np.arange(15) - 7
    g = np.exp(-(ax.astype(np.float64) ** 2) / (2.0 * 2.5 ** 2))
    return (g / g.sum())


def _phase_weights():
    g = _gaussian_sep()
    Wp = np.zeros((4, 5), np.float64)
    for p in range(4):
        for t in range(15):
            Wp[p, (p + t - 7) // 4 + 2] += g[t]
    return Wp.astype(np.float32)


def _band_matrices():
    # bands[t][v, y]: out_row(128t+y) = sum_v band[v, y] * hp[v]
    Wp = _phase_weights()
    bands = np.zeros((4, 128, 128), np.float32)
    for t in range(4):
        for y in range(128):
            yg = 128 * t + y
            v0, q = yg // 4, yg % 4
            for d in range(5):
                v = v0 + d - 2
                if 0 <= v < 128:
                    bands[t, v, y] = Wp[q, d]
    return bands


def _host_inputs(frame1, frame2):
    """Build the 8 per-core input maps."""
    bands = _band_matrices()
    in_maps = []
    for b in range(B):
        for w in range(2):
            xbase = 256 * w - 16
            sl1 = np.zeros((3, H, XL), np.float32)
            sl2 = np.zeros((3, H, XL), np.float32)
            lo, hi = max(0, xbase), min(W, xbase + XL)
            sl1[:, :, lo - xbase:hi - xbase] = frame1[b][:, :, lo:hi]
            sl2[:, :, lo - xbase:hi - xbase] = frame2[b][:, :, lo:hi]
            # interleave to [v=128, ry=4, c=3, x] so the load is one
            # contiguous 13.8KB run per partition
            il1 = np.ascontiguousarray(
                sl1.reshape(3, 128, 4, XL).transpose(1, 2, 0, 3))
            il2 = np.ascontiguousarray(
                sl2.reshape(3, 128, 4, XL).transpose(1, 2, 0, 3))
            # column-validity mask for the gray feature plane (ry-replicated)
            xcols = xbase + np.arange(XL)
            valid = (xcols >= 0) & (xcols < W)
            xm = np.where(valid, POS, NEG).astype(np.float32)
            xmask = np.tile(np.tile(xm, 4)[None, :], (128, 1))
            # anchor-validity mask
            gxg = 64 * w - 2 + np.arange(GXL)
            gm = ((gxg >= 0) & (gxg < 128)).astype(np.float32)
            gmask = np.tile(gm[None, :], (128, 1))
            ws = np.repeat((15.0 - np.arange(15, dtype=np.float32)),
                           GXL).reshape(1, 15 * GXL)
            consts = np.concatenate(
                [xmask, gmask,
                 bands.transpose(1, 0, 2).reshape(128, 512),
                 np.tile(ws, (128, 1))], axis=1)
            in_maps.append({"f1s": il1, "f2s": il2, "consts": consts})
    return in_maps


# ----------------------------------------------------------------------------
# device program
# ----------------------------------------------------------------------------

def _emit_features(nc, raw, feat, scr, full_v=True):
    """gray + sobel into feat [128, ry4, c3, XL] from raw [128, ry4, c3, XL].

    feat c0=gray, c1=fx, c2=fy. full_v: all 4 row phases (frame2) or ry=0.
    """
    g = feat[:][:, :, 0, :]                      # [128, 4, XL] gray plane
    # gray = (0.299 R + 0.587 G) + 0.114 B
    nc.vector.tensor_scalar_mul(g, raw[:][:, :, 0, :], 0.299)
    nc.vector.scalar_tensor_tensor(g, raw[:][:, :, 1, :], 0.587, g,
                                   Alu.mult, Alu.add)
    nc.vector.scalar_tensor_tensor(g, raw[:][:, :, 2, :], 0.114, g,
                                   Alu.mult, Alu.add)

    sd = scr["sd"]                               # [128, ry4, 2, XL] s=0, d=1
    s = sd[:][:, :, 0, :]
    d = sd[:][:, :, 1, :]
    # horizontal pass on interior columns [1, XL-1)
    gl = g[:, :, 0:XL - 2]
    gc = g[:, :, 1:XL - 1]
    gr = g[:, :, 2:XL]
    nc.vector.tensor_tensor(d[:, :, 1:XL - 1], gr, gl, Alu.subtract)
    nc.vector.scalar_tensor_tensor(s[:, :, 1:XL - 1], gc, 2.0, gl,
                                   Alu.mult, Alu.add)
    nc.vector.tensor_tensor(s[:, :, 1:XL - 1], s[:, :, 1:XL - 1], gr,
                            Alu.add)
    # zero the unused boundary columns so nothing downstream sees junk
    nc.vector.memset(sd[:][:, :, :, 0:1], 0.0)
    nc.vector.memset(sd[:][:, :, :, XL - 1:XL], 0.0)

    # vertical pass: row r = 4v+ry; cross-phase neighbors via shifted copies
    # sdm1[v] = sd[v-1, ry=3]  (s3s/d3s), sdp1[v] = sd[v+1, ry=0] (s0s/d0s)
    sdm1 = scr["sdm1"]                           # [128, 2, XL]
    touch = scr["touch"]
    nc.vector.memset(sdm1[:][0:1, :, :], 0.0)
    nc.gpsimd.dma_start(sdm1[:][1:64, :, :], sd[:][0:63, 3, :, :])
    nc.gpsimd.dma_start(sdm1[:][64:128, :, :], sd[:][63:127, 3, :, :])
    fxp = feat[:][:, :, 1, :]
    fyp = feat[:][:, :, 2, :]

    def vconv(ry, dm1, dp1, sm1, sp1):
        # fx[ry] = dm1 + 2*d[ry] + dp1 ; fy[ry] = sp1 - sm1
        nc.vector.scalar_tensor_tensor(fxp[:, ry, :], d[:, ry, :], 2.0, dm1,
                                       Alu.mult, Alu.add)
        nc.vector.tensor_tensor(fxp[:, ry, :], fxp[:, ry, :], dp1, Alu.add)
        nc.vector.tensor_tensor(fyp[:, ry, :], sp1, sm1, Alu.subtract)

    if full_v:
        sdp1 = scr["sdp1"]
        # pre-zero the tail so the DMA leaves partition 127 = 0 (row 512)
        nc.vector.memset(sdp1[:][96:128, :, :], 0.0)
        nc.gpsimd.dma_start(sdp1[:][0:64, :, :], sd[:][1:65, 0, :, :])
        nc.gpsimd.dma_start(sdp1[:][64:127, :, :], sd[:][65:128, 0, :, :])
        vconv(1, d[:, 0, :], d[:, 2, :], s[:, 0, :], s[:, 2, :])
        vconv(2, d[:, 1, :], d[:, 3, :], s[:, 1, :], s[:, 3, :])
        vconv(3, d[:, 2, :], sdp1[:][:, 1, :], s[:, 2, :], sdp1[:][:, 0, :])
    vconv(0, sdm1[:][:, 1, :], d[:, 1, :], sdm1[:][:, 0, :], s[:, 1, :])


def build_program():
    nc = bacc.Bacc("TRN2", target_bir_lowering=False, debug=False)

    f1s_d = nc.dram_tensor("f1s", [128, 4, 3, XL], F32, kind="ExternalInput")
    f2s_d = nc.dram_tensor("f2s", [128, 4, 3, XL], F32, kind="ExternalInput")
    consts_d = nc.dram_tensor("consts",
                              [128, 4 * XL + GXL + 512 + 15 * GXL], F32,
                              kind="ExternalInput")
    out_d = nc.dram_tensor("out", [128, 4, 2, 256], F32,
                           kind="ExternalOutput")

    with tile.TileContext(nc) as tc:
        with tc.tile_pool(name="main", bufs=1) as pool, \
             tc.tile_pool(name="psum", bufs=4, space="PSUM") as psum_pool:

            raw1 = pool.tile([128, 4, 3, XL], F32)
            raw2 = pool.tile([128, 4, 3, XL], F32)
            feat1 = pool.tile([128, 4, 3, XL], F32)
            feat2 = pool.tile([128, 4, 3, XL], F32)
            sd_scr = pool.tile([128, 4, 2, XL], F32)
            sdm1 = pool.tile([128, 2, XL], F32)
            sdp1 = pool.tile([128, 2, XL], F32)
            sdm1f1 = pool.tile([128, 2, XL], F32)
            consts = pool.tile(
                [128, 4 * XL + GXL + 512 + 15 * GXL], F32)
            bands2 = pool.tile([128, 4, 128], F32)
            q_t = pool.tile([128, 4, XL], F32)
            r0_t = pool.tile([128, 4, XL], F32)
            a_t = pool.tile([128, 4, XL], F32)
            f2px = pool.tile([128, 16, 3, XL], F32)
            # aliases raw2's slot (raw2 is dead before correlation starts)
            prod = pool.tile([128, 3, 16, GXL], F32, tag="raw2")
            s2_t = pool.tile([128, 16, GXL], F32)
            corr16 = pool.tile([128, 16, GXL], F32)
            rowmax = pool.tile([128, 16, GXL], F32)
            colmax = pool.tile([128, 15, GXL], F32)
            wsum = pool.tile([128, 15, GXL], F32)
            m_t = pool.tile([128, GXL], F32)
            fm_t = pool.tile([128, GXL], F32)
            grid = pool.tile([128, 2, GXL], F32)
            hp = pool.tile([128, 2, 256], F32)
            hsc = pool.tile([128, 2, 64], F32)
            tscr = pool.tile([128, 64], F32)
            # V-pass tiles; alias slots of f2-norm scratch (dead by then)
            smsb = [pool.tile([128, 2, 256], F32, name=f"smsb{t}", tag=tg)
                    for t, tg in enumerate(("q_t", "r0_t", "a_t", "sd_scr"))]
            nq1s = [pool.tile([128, 256], F32, name=f"nq1_{t}")
                    for t in range(4)]
            nq2s = [pool.tile([128, 256], F32, name=f"nq2_{t}")
                    for t in range(4)]
            nrs = [pool.tile([128, 256], F32, name=f"nr_{t}")
                   for t in range(4)]
            nms = [pool.tile([128, 256], F32, name=f"nm_{t}")
                   for t in range(4)]
            # slots 8-11 channel sums for all dx, computed from feat2
            # during the f2px DMA flight (aliases the dead raw1 slot)
            corrA = pool.tile([128, 15, 4, GXL], F32, tag="raw1")
            rowmaxA = pool.tile([128, 4, GXL], F32)
            # one output staging tile -> single out DMA (aliases raw1)
            outsb = pool.tile([128, 4, 2, 256], F32, tag="raw1")

            _touch_n = [0]

            def touch(ap):
                # one-wait funnel: absorb a DMA-queue semaphore into the
                # DVE engine clock so consumers carry fewer sync waits
                k = _touch_n[0] = _touch_n[0] + 1
                nc.vector.tensor_copy(tscr[:][32:33, k % 64:k % 64 + 1], ap)

            # ---------------- input DMAs ----------------
            # split across partition ranges -> parallel DMA engines
            for p0 in range(0, 128, 32):
                nc.sync.dma_start(raw2[:][p0:p0 + 32], f2s_d.ap()[p0:p0 + 32])
            for p0 in range(0, 128, 32):
                nc.sync.dma_start(raw1[:][p0:p0 + 32], f1s_d.ap()[p0:p0 + 32])
            nc.sync.dma_start(consts[:], consts_d.ap())
            touch(consts[:][32:33, 0:1])
            xmask = consts[:][:, 0:4 * XL].rearrange("p (r x) -> p r x", r=4)
            gmask = consts[:][:, 4 * XL:4 * XL + GXL]
            bands = consts[:][:, 4 * XL + GXL:4 * XL + GXL + 512].rearrange(
                "p (t y) -> p t y", t=4)
            wslot = consts[:][:, 4 * XL + GXL + 512:].rearrange(
                "p (s g) -> p s g", s=15)

            scr = {"sd": sd_scr, "sdm1": sdm1, "sdp1": sdp1, "touch": touch}

            # ---------------- frame2 features + normalize ----------------
            _emit_features(nc, raw2, feat2, scr, full_v=True)

            # q = (f0^2 + f1^2) + f2^2, clamped; r = rsqrt(q) via ACT
            # abs_reciprocal_sqrt + one DVE Newton step (all ACT funcs in
            # this kernel live in the abs_reciprocal_sqrt_and_small set)
            nc.scalar.activation(q_t[:], feat2[:][:, :, 0, :], Act.Square)
            nc.scalar.activation(r0_t[:], feat2[:][:, :, 1, :], Act.Square)
            nc.scalar.activation(a_t[:], feat2[:][:, :, 2, :], Act.Square)
            nc.vector.tensor_tensor(q_t[:], q_t[:], r0_t[:], Alu.add)
            nc.vector.tensor_tensor(q_t[:], q_t[:], a_t[:], Alu.add)
            nc.vector.tensor_scalar_max(q_t[:], q_t[:], 1e-24)
            nc.scalar.activation(r0_t[:], q_t[:], Act.Abs_reciprocal_sqrt)
            # Newton: r1 = r0*(1.5 - 0.5*q*r0^2)
            nc.vector.tensor_tensor(a_t[:], r0_t[:], r0_t[:], Alu.mult)
            nc.vector.tensor_tensor(a_t[:], a_t[:], q_t[:], Alu.mult)
            nc.vector.tensor_scalar(a_t[:], a_t[:], -0.5, 1.5, Alu.mult,
                                    Alu.add)
            nc.vector.tensor_tensor(r0_t[:], r0_t[:], a_t[:], Alu.mult)
            for c in range(3):
                nc.vector.tensor_tensor(feat2[:][:, :, c, :],
                                        feat2[:][:, :, c, :],
                                        r0_t[:], Alu.mult)
            # column-validity mask on the gray plane
            nc.vector.tensor_tensor(feat2[:][:, :, 0, :],
                                    feat2[:][:, :, 0, :],
                                    xmask, Alu.min)

            # ---------------- f2px replication ----------------
            # out-of-image corners first (overwritten where valid below)
            nc.vector.memset(f2px[:][0:2, 0:8, 0, :], float(NEG))
            nc.vector.memset(f2px[:][0:2, 0:8, 1:3, :], 0.0)
            nc.vector.memset(f2px[:][96:128, 12:16, 0, :], float(NEG))
            nc.vector.memset(f2px[:][96:128, 12:16, 1:3, :], 0.0)
            # slots s=4*ovi+ry take rows 4(p+ov)+ry; contiguous 13.8KB runs,
            # pieces spread over both HWDGE and SWDGE queues in parallel
            for ovi in (0, 1, 3):
                ov = ovi - 2
                p0, p1 = max(0, -ov), min(128, 128 - ov)
                for q0 in range(0, 128, 32):
                    a, b = max(p0, q0), min(p1, q0 + 32)
                    if a < b:
                        # HWDGE only: SWDGE descriptor generation is the
                        # straggler when pieces alternate onto gpsimd queues
                        nc.sync.dma_start(
                            f2px[:][a:b, 4 * ovi:4 * ovi + 4, :, :],
                            feat2[:][a + ov:b + ov, :, :, :])

            # ---------------- frame1 features (anchors only) ----------------
            scr1 = dict(scr)
            scr1["sdm1"] = sdm1f1
            _emit_features(nc, raw1, feat1, scr1, full_v=False)

            # ---------------- correlation ----------------
            # f1 anchors: ry=0 plane of feat1, cols 8+4j (no copy needed)
            f1v = feat1[:][:, 0, :, 8:8 + 4 * GXL - 3:4]      # [128, 3, 68]
            f1b4 = f1v.unsqueeze(1).broadcast_to([128, 4, 3, GXL])
            prodA = prod[:][:, :, 8:12, :].transpose([0, 2, 1, 3])
            # early phase: slots 8-11 (ov=0) read feat2 directly -> corrA,
            # overlapping the f2px replication DMAs
            for dx in range(15):
                f2vA = feat2[:][:, :, :, 1 + dx:1 + dx + 4 * GXL - 3:4]
                nc.vector.tensor_tensor(prodA, f1b4, f2vA, Alu.mult)
                nc.vector.tensor_tensor(s2_t[:][:, 8:12, :],
                                        prod[:][:, 0, 8:12, :],
                                        prod[:][:, 1, 8:12, :], Alu.add)
                nc.vector.tensor_tensor(corrA[:][:, dx, :, :],
                                        s2_t[:][:, 8:12, :],
                                        prod[:][:, 2, 8:12, :], Alu.add)
                nc.vector.tensor_reduce(
                    colmax[:][:, dx, :],
                    corrA[:][:, dx, :, :].transpose([0, 2, 1]),
                    axis=AX.X, op=Alu.max)
                if dx == 0:
                    nc.vector.tensor_copy(rowmaxA[:], corrA[:][:, 0, :, :])
                else:
                    nc.vector.tensor_tensor(rowmaxA[:], rowmaxA[:],
                                            corrA[:][:, dx, :, :], Alu.max)
            f1b7 = f1v.unsqueeze(1).broadcast_to([128, 7, 3, GXL])
            prod7 = prod[:][:, :, 1:8, :].transpose([0, 2, 1, 3])
            prod4 = prod[:][:, :, 12:16, :].transpose([0, 2, 1, 3])
            for dx in range(15):
                xs = slice(1 + dx, 1 + dx + 4 * GXL - 3, 4)
                nc.vector.tensor_tensor(prod7, f1b7,
                                        f2px[:][:, 1:8, :, xs], Alu.mult)
                nc.vector.tensor_tensor(prod4, f1b4,
                                        f2px[:][:, 12:16, :, xs], Alu.mult)
                nc.vector.tensor_tensor(s2_t[:][:, 1:8, :],
                                        prod[:][:, 0, 1:8, :],
                                        prod[:][:, 1, 1:8, :], Alu.add)
                nc.vector.tensor_tensor(corr16[:][:, 1:8, :],
                                        s2_t[:][:, 1:8, :],
                                        prod[:][:, 2, 1:8, :], Alu.add)
                nc.vector.tensor_tensor(s2_t[:][:, 12:16, :],
                                        prod[:][:, 0, 12:16, :],
                                        prod[:][:, 1, 12:16, :], Alu.add)
                nc.vector.tensor_tensor(corr16[:][:, 12:16, :],
                                        s2_t[:][:, 12:16, :],
                                        prod[:][:, 2, 12:16, :], Alu.add)
                # colmax[dx] = max(early slots 8-11, late slots 1-7,12-15)
                nc.vector.tensor_reduce(
                    m_t[:], corr16[:][:, 1:8, :].transpose([0, 2, 1]),
                    axis=AX.X, op=Alu.max)
                nc.vector.tensor_tensor(colmax[:][:, dx, :],
                                        colmax[:][:, dx, :], m_t[:], Alu.max)
                nc.vector.tensor_reduce(
                    m_t[:], corr16[:][:, 12:16, :].transpose([0, 2, 1]),
                    axis=AX.X, op=Alu.max)
                nc.vector.tensor_tensor(colmax[:][:, dx, :],
                                        colmax[:][:, dx, :], m_t[:], Alu.max)
                if dx == 0:
                    nc.vector.tensor_copy(rowmax[:][:, 1:8, :],
                                          corr16[:][:, 1:8, :])
                    nc.vector.tensor_copy(rowmax[:][:, 12:16, :],
                                          corr16[:][:, 12:16, :])
                else:
                    nc.vector.tensor_tensor(rowmax[:][:, 1:8, :],
                                            rowmax[:][:, 1:8, :],
                                            corr16[:][:, 1:8, :], Alu.max)
                    nc.vector.tensor_tensor(rowmax[:][:, 12:16, :],
                                            rowmax[:][:, 12:16, :],
                                            corr16[:][:, 12:16, :], Alu.max)
            nc.vector.tensor_copy(rowmax[:][:, 8:12, :], rowmaxA[:])

            # ---------------- argmax -> displacement grid ----------------
            def first_argmax(maxbuf, ch, s_lo):
                nc.vector.tensor_reduce(
                    m_t[:], maxbuf[:][:, s_lo:s_lo + 15, :]
                    .transpose([0, 2, 1]), axis=AX.X, op=Alu.max)
                mb = m_t[:].unsqueeze(1).broadcast_to([128, 15, GXL])
                nc.vector.tensor_tensor(
                    wsum[:], maxbuf[:][:, s_lo:s_lo + 15, :], mb, Alu.is_ge)
                nc.vector.tensor_tensor(wsum[:], wsum[:], wslot, Alu.mult)
                nc.vector.tensor_reduce(
                    fm_t[:], wsum[:].transpose([0, 2, 1]), axis=AX.X,
                    op=Alu.max)
                # disp = (argmax-7)/512 = (8 - fm)/512 ; zero invalid anchors
                nc.vector.tensor_scalar(fm_t[:], fm_t[:], -1.0 / 512.0,
                                        8.0 / 512.0, Alu.mult, Alu.add)
                nc.vector.tensor_tensor(grid[:][:, ch, :], fm_t[:], gmask,
                                        Alu.mult)

            first_argmax(rowmax, 1, 1)   # dy -> channel 1
            first_argmax(colmax, 0, 0)   # dx -> channel 0

            # ---------------- smoothing H-pass (phase weights) -------------
            Wp = _phase_weights()
            hsc2 = hsc[:]
            for p in range(4):
                nc.vector.tensor_scalar_mul(
                    hsc2, grid[:][:, :, 0:64], float(Wp[p, 0]))
                for dd in range(1, 4):
                    nc.vector.scalar_tensor_tensor(
                        hsc2, grid[:][:, :, dd:dd + 64],
                        float(Wp[p, dd]), hsc2, Alu.mult, Alu.add)
                nc.vector.scalar_tensor_tensor(
                    hp[:][:, :, p:256:4], grid[:][:, :, 4:4 + 64],
                    float(Wp[p, 4]), hsc2, Alu.mult, Alu.add)

            # ---------------- V-pass (PE banded matmul) + normalize --------
            # route bands through DVE so matmuls carry a single (DVE) wait
            nc.vector.tensor_copy(bands2[:], bands)
            # ACT funnel: advance ACT's view of the DVE clock past hp/grid
            nc.scalar.copy(tscr[:][32:33, 0:1], hp[:][32:33, 0, 0:1])
            rhs = hp[:].rearrange("p c x -> p (c x)")
            for t in range(4):
                ps = psum_pool.tile([128, 512], F32, tag="vps")
                nc.tensor.matmul(ps[:], bands2[:][:, t, :], rhs,
                                 start=True, stop=True)
                # single PSUM reader: ACT copies to SBUF, rest reads SBUF
                sm = smsb[t]
                nc.scalar.copy(sm[:].rearrange("p c x -> p (c x)"), ps[:])
                psv = sm[:]
                nq1, nq2, nr, nm = nq1s[t], nq2s[t], nrs[t], nms[t]
                nc.scalar.activation(nq1[:], psv[:, 0, :], Act.Square)
                nc.scalar.activation(nq2[:], psv[:, 1, :], Act.Square)
                # q = max(qx,1e-30)+qy ; mag = q * rsqrt(q)
                nc.vector.scalar_tensor_tensor(nq1[:], nq1[:], 1e-30, nq2[:],
                                               Alu.max, Alu.add)
                nc.scalar.activation(nr[:], nq1[:], Act.Abs_reciprocal_sqrt)
                # Newton on rsqrt(q), then mag = q*r
                nc.vector.tensor_tensor(nm[:], nr[:], nr[:], Alu.mult)
                nc.vector.tensor_tensor(nm[:], nm[:], nq1[:], Alu.mult)
                nc.vector.tensor_scalar(nm[:], nm[:], -0.5, 1.5, Alu.mult,
                                        Alu.add)
                nc.vector.tensor_tensor(nr[:], nr[:], nm[:], Alu.mult)
                nc.vector.tensor_tensor(nm[:], nq1[:], nr[:], Alu.mult)
                # magc = max(mag,1e-6)+1e-6 ; 1/magc = ars(magc^2) + Newton
                nc.vector.tensor_scalar(nm[:], nm[:], 1e-6, 1e-6, Alu.max,
                                        Alu.add)
                nc.vector.tensor_tensor(nq2[:], nm[:], nm[:], Alu.mult)
                nc.scalar.activation(nr[:], nq2[:], Act.Abs_reciprocal_sqrt)
                nc.vector.tensor_tensor(nm[:], nr[:], nr[:], Alu.mult)
                nc.vector.tensor_tensor(nm[:], nm[:], nq2[:], Alu.mult)
                nc.vector.tensor_scalar(nm[:], nm[:], -0.5, 1.5, Alu.mult,
                                        Alu.add)
                nc.vector.tensor_tensor(nr[:], nr[:], nm[:], Alu.mult)
                nc.vector.tensor_tensor(outsb[:][:, t, 0, :], psv[:, 0, :],
                                        nr[:], Alu.mult)
                nc.vector.tensor_tensor(outsb[:][:, t, 1, :], psv[:, 1, :],
                                        nr[:], Alu.mult)
            for p0 in range(0, 128, 32):
                nc.sync.dma_start(out_d.ap()[p0:p0 + 32],
                                  outsb[:][p0:p0 + 32])

    nc.compile()
    return nc


_NC_CACHE = None


def _get_nc():
    global _NC_CACHE
    if _NC_CACHE is None:
        _NC_CACHE = build_program()
    return _NC_CACHE


def kernel(frame1, frame2):
    frame1 = np.asarray(frame1, dtype=np.float32)
    frame2 = np.asarray(frame2, dtype=np.float32)
    nc = _get_nc()
    in_maps = _host_inputs(frame1, frame2)
    res = run_bass_kernel_spmd(nc, in_maps, core_ids=list(range(8)))
    if res.exec_time_ns is not None:
        print(f"HW exec time: {res.exec_time_ns} ns")
    out = np.empty((B, 2, H, W), np.float32)
    for b in range(B):
        for w in range(2):
            o = res.results[2 * b + w]["out"]        # [128, 4, 2, 256]
            o = o.transpose(2, 1, 0, 3).reshape(2, H, 256)
            out[b, :, :, 256 * w:256 * w + 256] = o
    return out



# revision 6
# speedup vs baseline: 1.2725x; 1.2725x over previous
"""Dense optical flow kernel for Trainium2, 8-core SPMD.

Pipeline (per core = one (sample, x-half) pair), x-polyphase layout
(x = 4j + p) so every correlation window read is a dense stride-1 run:

  frames -> gray/sobel features (row-polyphase ry, col-polyphase p)
  -> l2-normalize f2 (ACT abs_reciprocal_sqrt + DVE Newton)
  -> replicated window tensor f2px -> 15x15 windowed correlation (f32)
    on DVE with dense mults/adds and dense pairwise max trees
  -> first-argmax -> displacement grid -> separable gaussian smoothing
    (phase H-pass on DVE, banded-matmul V-pass on PE)
  -> direction normalize (ACT-heavy, no Newton) -> full-res flow.

Cross-partition row shifts for the vertical sobel go through PE
shift-matmuls (off-diagonal identity) instead of SBUF->SBUF DMA.
"""

import numpy as np

import concourse.bacc as bacc
import concourse.tile as tile
from concourse import mybir
from concourse.bass_utils import run_bass_kernel_spmd

F32 = mybir.dt.float32
Alu = mybir.AluOpType
Act = mybir.ActivationFunctionType
AX = mybir.AxisListType

H = 512
W = 512
B = 4
XL = 288          # per-core padded column span
JL = 72           # XL / 4 (x-polyphase)
GXL = 68          # local anchor columns (64 + 2 halo each side)
NEG = np.float32(-1.0e30)
POS = np.float32(3.0e38)

# consts layout offsets (fp32 elements per partition)
OFF_XMASK = 0                   # [4ry, 4p, 72j] = 1152
OFF_GMASK = 1152                # [68]
OFF_BANDS = 1220                # [4t, 128y] = 512
OFF_WS = 1732                   # [15, 68] = 1020
OFF_SHDN = 2752                 # [128]
OFF_SHUP = 2880                 # [128]
NCONST = 3008


# ----------------------------------------------------------------------------
# constants (host side)
# ----------------------------------------------------------------------------

def _gaussian_sep():
    ax = np.arange(15) - 7
    g = np.exp(-(ax.astype(np.float64) ** 2) / (2.0 * 2.5 ** 2))
    return (g / g.sum())


def _phase_weights():
    g = _gaussian_sep()
    Wp = np.zeros((4, 5), np.float64)
    for p in range(4):
        for t in range(15):
            Wp[p, (p + t - 7) // 4 + 2] += g[t]
    return Wp.astype(np.float32)


def _band_matrices():
    # bands[t][v, y]: out_row(128t+y) = sum_v band[v, y] * hp[v]
    Wp = _phase_weights()
    bands = np.zeros((4, 128, 128), np.float32)
    for t in range(4):
        for y in range(128):
            yg = 128 * t + y
            v0, q = yg // 4, yg % 4
            for d in range(5):
                v = v0 + d - 2
                if 0 <= v < 128:
                    bands[t, v, y] = Wp[q, d]
    return bands


def _poly(a):
    """[..., 288] -> [..., 4p, 72j] with x = 4j + p."""
    return np.ascontiguousarray(
        a.reshape(*a.shape[:-1], JL, 4).swapaxes(-1, -2))


def _host_inputs(frame1, frame2):
    """Build the 8 per-core input maps."""
    bands = _band_matrices()
    # PE shift matrices: out[i] = sum_p lhsT[p, i] * in[p]
    shdn = np.zeros((128, 128), np.float32)   # out[i] = in[i-1]
    shup = np.zeros((128, 128), np.float32)   # out[i] = in[i+1]
    for i in range(1, 128):
        shdn[i - 1, i] = 1.0
        shup[i, i - 1] = 1.0
    in_maps = []
    for b in range(B):
        for w in range(2):
            xbase = 256 * w - 16
            sl1 = np.zeros((3, H, XL), np.float32)
            sl2 = np.zeros((3, H, XL), np.float32)
            lo, hi = max(0, xbase), min(W, xbase + XL)
            sl1[:, :, lo - xbase:hi - xbase] = frame1[b][:, :, lo:hi]
            sl2[:, :, lo - xbase:hi - xbase] = frame2[b][:, :, lo:hi]
            # [3c, 512, 288] -> [128v, 4ry, 3c, 4p, 72j]
            il1 = np.ascontiguousarray(
                _poly(sl1).reshape(3, 128, 4, 4, JL).transpose(1, 2, 0, 3, 4))
            il2 = np.ascontiguousarray(
                _poly(sl2).reshape(3, 128, 4, 4, JL).transpose(1, 2, 0, 3, 4))
            # column-validity mask for the gray plane, polyphase, ry-tiled
            xcols = xbase + np.arange(XL)
            valid = (xcols >= 0) & (xcols < W)
            xm = _poly(np.where(valid, POS, NEG).astype(np.float32))  # [4,72]
            xmask = np.tile(xm.reshape(-1), 4)                        # ry x 4
            # anchor-validity mask
            gxg = 64 * w - 2 + np.arange(GXL)
            gm = ((gxg >= 0) & (gxg < 128)).astype(np.float32)
            ws = np.repeat((15.0 - np.arange(15, dtype=np.float32)),
                           GXL)
            row = np.concatenate([xmask, gm, np.zeros(512, np.float32),
                                  ws, np.zeros(256, np.float32)])
            consts = np.tile(row[None, :], (128, 1))
            consts[:, OFF_BANDS:OFF_BANDS + 512] = \
                bands.transpose(1, 0, 2).reshape(128, 512)
            consts[:, OFF_SHDN:OFF_SHDN + 128] = shdn
            consts[:, OFF_SHUP:OFF_SHUP + 128] = shup
            in_maps.append({"f1s": il1.reshape(128, -1),
                            "f2s": il2.reshape(128, -1),
                            "consts": consts})
    return in_maps


# ----------------------------------------------------------------------------
# device program
# ----------------------------------------------------------------------------

def build_program():
    nc = bacc.Bacc("TRN2", target_bir_lowering=False, debug=False)

    f1s_d = nc.dram_tensor("f1s", [128, 4 * 3 * 4 * JL], F32,
                           kind="ExternalInput")
    f2s_d = nc.dram_tensor("f2s", [128, 4 * 3 * 4 * JL], F32,
                           kind="ExternalInput")
    consts_d = nc.dram_tensor("consts", [128, NCONST], F32,
                              kind="ExternalInput")
    out_d = nc.dram_tensor("out", [128, 4, 2, 256], F32,
                           kind="ExternalOutput")

    TT = None  # set below

    with tile.TileContext(nc) as tc:
        with tc.tile_pool(name="main", bufs=1) as pool, \
             tc.tile_pool(name="psum", bufs=4, space="PSUM") as psum_pool:

            raw1 = pool.tile([128, 4, 3, 4, JL], F32)
            raw2 = pool.tile([128, 4, 3, 4, JL], F32)
            feat2 = pool.tile([128, 4, 3, 4, JL], F32)
            gray1 = pool.tile([128, 4, 4, JL], F32)
            sd = pool.tile([128, 4, 2, 4, JL], F32)     # s=0, d=1
            sd1 = pool.tile([128, 4, 2, JL], F32)       # f1, phase 0 only
            q_t = pool.tile([128, 4, 4, JL], F32)
            r0_t = pool.tile([128, 4, 4, JL], F32)
            a_t = pool.tile([128, 4, 4, JL], F32)
            consts = pool.tile([128, NCONST], F32)
            f2px = pool.tile([128, 16, 3, 4, JL], F32)
            f1a = pool.tile([128, 3, JL], F32)
            # correlation scratch; aliases of tensors dead by corr time
            prod = pool.tile([128, 16, 3, GXL], F32, tag="raw2")
            s2_t = pool.tile([128, 16, GXL], F32, tag="a_t")
            c16 = pool.tile([128, 2, 16, GXL], F32, tag="sd")
            cE = pool.tile([128, 2, 4, GXL], F32)
            m8 = pool.tile([128, 2, 8, GXL], F32)
            rowmax = pool.tile([128, 16, GXL], F32, tag="q_t")
            colmax = pool.tile([128, 15, GXL], F32, tag="r0_t")
            wsum = pool.tile([128, 15, GXL], F32)
            m_t = pool.tile([128, GXL], F32)
            fm_t = pool.tile([128, GXL], F32)
            grid = pool.tile([128, 2, GXL], F32)
            hp = pool.tile([128, 2, 256], F32)
            hsc = pool.tile([128, 2, 64], F32)
            tscr = pool.tile([128, 64], F32)
            bands2 = pool.tile([128, 4, 128], F32)
            # V-pass scratch aliases f2px (dead after main corr loop)
            vps = pool.tile([128, 4, 8, 256], F32, tag="f2px")
            outsb = pool.tile([128, 4, 2, 256], F32, tag="raw1")

            TT = nc.vector.tensor_tensor
            STT = nc.vector.scalar_tensor_tensor
            _touch_n = [0]

            def touch(ap):
                k = _touch_n[0] = _touch_n[0] + 1
                nc.vector.tensor_copy(tscr[:][32:33, k % 64:k % 64 + 1], ap)

            # ---------------- input DMAs ----------------
            for p0 in range(0, 128, 32):
                nc.sync.dma_start(
                    raw2[:].rearrange("p r c q j -> p (r c q j)")[p0:p0 + 32],
                    f2s_d.ap()[p0:p0 + 32])
            for p0 in range(0, 128, 32):
                nc.sync.dma_start(
                    raw1[:].rearrange("p r c q j -> p (r c q j)")[p0:p0 + 32],
                    f1s_d.ap()[p0:p0 + 32])
            nc.sync.dma_start(consts[:], consts_d.ap())
            touch(consts[:][32:33, 0:1])
            xmask = consts[:][:, OFF_XMASK:OFF_XMASK + 1152].rearrange(
                "p (r q j) -> p r q j", r=4, q=4)
            gmask = consts[:][:, OFF_GMASK:OFF_GMASK + GXL]
            bands = consts[:][:, OFF_BANDS:OFF_BANDS + 512].rearrange(
                "p (t y) -> p t y", t=4)
            wslot = consts[:][:, OFF_WS:OFF_WS + 15 * GXL].rearrange(
                "p (s g) -> p s g", s=15)
            shdn = consts[:][:, OFF_SHDN:OFF_SHDN + 128]
            shup = consts[:][:, OFF_SHUP:OFF_SHUP + 128]

            # f2px out-of-image corners (overwritten where valid by DMA);
            # gpsimd memsets overlap the input DMA wait
            nc.gpsimd.memset(f2px[:][0:2, 0:8, 0, :, :], float(NEG))
            nc.gpsimd.memset(f2px[:][0:2, 0:8, 1:3, :, :], 0.0)
            nc.gpsimd.memset(f2px[:][96:128, 12:16, 0, :, :], float(NEG))
            nc.gpsimd.memset(f2px[:][96:128, 12:16, 1:3, :, :], 0.0)

            # ---------------- frame2 features ----------------
            g2 = feat2[:][:, :, 0, :, :]                 # [128, 4ry, 4p, 72]
            r2v = raw2[:]
            nc.vector.tensor_scalar_mul(g2, r2v[:, :, 0, :, :], 0.299)
            STT(g2, r2v[:, :, 1, :, :], 0.587, g2, Alu.mult, Alu.add)
            STT(g2, r2v[:, :, 2, :, :], 0.114, g2, Alu.mult, Alu.add)
            # gray^2 on ACT, overlapped with the H-pass below
            nc.scalar.activation(q_t[:], g2, Act.Square)

            s_ = sd[:][:, :, 0, :, :]                    # [128, 4ry, 4p, 72]
            d_ = sd[:][:, :, 1, :, :]
            # d = g(x+1) - g(x-1); s = g(x-1) + 2 g(x) + g(x+1)
            TT(d_[:, :, 1:3, :], g2[:, :, 2:4, :], g2[:, :, 0:2, :],
               Alu.subtract)
            TT(d_[:, :, 0, 1:JL], g2[:, :, 1, 1:JL], g2[:, :, 3, 0:JL - 1],
               Alu.subtract)
            TT(d_[:, :, 3, 0:JL - 1], g2[:, :, 0, 1:JL], g2[:, :, 2, 0:JL - 1],
               Alu.subtract)
            STT(s_[:, :, 1:3, :], g2[:, :, 1:3, :], 2.0, g2[:, :, 0:2, :],
                Alu.mult, Alu.add)
            TT(s_[:, :, 1:3, :], s_[:, :, 1:3, :], g2[:, :, 2:4, :], Alu.add)
            STT(s_[:, :, 0, 1:JL], g2[:, :, 0, 1:JL], 2.0,
                g2[:, :, 3, 0:JL - 1], Alu.mult, Alu.add)
            TT(s_[:, :, 0, 1:JL], s_[:, :, 0, 1:JL], g2[:, :, 1, 1:JL],
               Alu.add)
            STT(s_[:, :, 3, 0:JL - 1], g2[:, :, 3, 0:JL - 1], 2.0,
                g2[:, :, 2, 0:JL - 1], Alu.mult, Alu.add)
            TT(s_[:, :, 3, 0:JL - 1], s_[:, :, 3, 0:JL - 1],
               g2[:, :, 0, 1:JL], Alu.add)
            # x boundary columns (x=0 and x=287)
            nc.vector.memset(sd[:][:, :, :, 0, 0:1], 0.0)
            nc.vector.memset(sd[:][:, :, :, 3, JL - 1:JL], 0.0)

            # cross-partition row shifts on PE: dn -> row 4v-1, up -> row 4v+4
            ps_sm1 = psum_pool.tile([128, 288], F32, tag="shift")
            ps_dm1 = psum_pool.tile([128, 288], F32, tag="shift")
            ps_sp1 = psum_pool.tile([128, 288], F32, tag="shift")
            ps_dp1 = psum_pool.tile([128, 288], F32, tag="shift")
            sd_f = sd[:].rearrange("p r s q j -> p r s (q j)")
            nc.tensor.matmul(ps_sm1[:], shdn, sd_f[:, 3, 0, :],
                             start=True, stop=True)
            nc.tensor.matmul(ps_dm1[:], shdn, sd_f[:, 3, 1, :],
                             start=True, stop=True)
            nc.tensor.matmul(ps_sp1[:], shup, sd_f[:, 0, 0, :],
                             start=True, stop=True)
            nc.tensor.matmul(ps_dp1[:], shup, sd_f[:, 0, 1, :],
                             start=True, stop=True)

            fx2 = feat2[:][:, :, 1, :, :]
            fy2 = feat2[:][:, :, 2, :, :]

            def vconv2(ry, dm1, dp1, sm1, sp1):
                STT(fx2[:, ry, :, :], d_[:, ry, :, :], 2.0, dm1,
                    Alu.mult, Alu.add)
                TT(fx2[:, ry, :, :], fx2[:, ry, :, :], dp1, Alu.add)
                TT(fy2[:, ry, :, :], sp1, sm1, Alu.subtract)

            vconv2(1, d_[:, 0, :, :], d_[:, 2, :, :],
                   s_[:, 0, :, :], s_[:, 2, :, :])
            vconv2(2, d_[:, 1, :, :], d_[:, 3, :, :],
                   s_[:, 1, :, :], s_[:, 3, :, :])
            pq = lambda ps: ps[:].rearrange("p (q j) -> p q j", q=4)
            vconv2(0, pq(ps_dm1), d_[:, 1, :, :],
                   pq(ps_sm1), s_[:, 1, :, :])
            vconv2(3, d_[:, 2, :, :], pq(ps_dp1),
                   s_[:, 2, :, :], pq(ps_sp1))

            # ---------------- frame2 normalize ----------------
            nc.scalar.activation(r0_t[:], fx2, Act.Square)
            nc.scalar.activation(a_t[:], fy2, Act.Square)
            # q = max(fx2^2, 1e-24) + fy2^2 + gray^2
            STT(r0_t[:], r0_t[:], 1e-24, a_t[:], Alu.max, Alu.add)
            TT(q_t[:], q_t[:], r0_t[:], Alu.add)
            nc.scalar.activation(r0_t[:], q_t[:], Act.Abs_reciprocal_sqrt)
            # Newton: r1 = r0*(1.5 - 0.5*q*r0^2)
            TT(a_t[:], r0_t[:], r0_t[:], Alu.mult)
            TT(a_t[:], a_t[:], q_t[:], Alu.mult)
            nc.vector.tensor_scalar(a_t[:], a_t[:], -0.5, 1.5, Alu.mult,
                                    Alu.add)
            TT(r0_t[:], r0_t[:], a_t[:], Alu.mult)
            for c in range(3):
                TT(feat2[:][:, :, c, :, :], feat2[:][:, :, c, :, :],
                   r0_t[:], Alu.mult)
            TT(g2, g2, xmask, Alu.min)

            # rails for the corr max trees (aliased tensors now dead)
            nc.vector.memset(rowmax[:], float(NEG))
            nc.vector.memset(c16[:][:, :, 0:1, :], float(NEG))
            nc.vector.memset(c16[:][:, :, 8:12, :], float(NEG))

            # ---------------- f2px replication DMAs ----------------
            f2px_f = f2px[:].rearrange("p s c q j -> p s (c q j)")
            feat2_f = feat2[:].rearrange("p r c q j -> p r (c q j)")
            for ovi in (0, 1, 3):
                ov = ovi - 2
                p0, p1 = max(0, -ov), min(128, 128 - ov)
                for q0 in range(0, 128, 32):
                    a, b = max(p0, q0), min(p1, q0 + 32)
                    if a < b:
                        nc.sync.dma_start(
                            f2px_f[a:b, 4 * ovi:4 * ovi + 4, :],
                            feat2_f[a + ov:b + ov, :, :])

            # ---------------- frame1 features (anchors only) -------------
            g1 = gray1[:]
            r1v = raw1[:]
            nc.vector.tensor_scalar_mul(g1, r1v[:, :, 0, :, :], 0.299)
            STT(g1, r1v[:, :, 1, :, :], 0.587, g1, Alu.mult, Alu.add)
            STT(g1, r1v[:, :, 2, :, :], 0.114, g1, Alu.mult, Alu.add)
            s1 = sd1[:][:, :, 0, :]
            d1 = sd1[:][:, :, 1, :]
            # phase-0 H-pass only: d = g[p1,j] - g[p3,j-1]
            #                      s = g[p3,j-1] + 2 g[p0,j] + g[p1,j]
            TT(d1[:, :, 1:JL], g1[:, :, 1, 1:JL], g1[:, :, 3, 0:JL - 1],
               Alu.subtract)
            STT(s1[:, :, 1:JL], g1[:, :, 0, 1:JL], 2.0,
                g1[:, :, 3, 0:JL - 1], Alu.mult, Alu.add)
            TT(s1[:, :, 1:JL], s1[:, :, 1:JL], g1[:, :, 1, 1:JL], Alu.add)
            ps1 = psum_pool.tile([128, 2 * JL], F32, tag="shift")
            nc.tensor.matmul(ps1[:], shdn,
                             sd1[:][:, 3, :, :].rearrange("p s j -> p (s j)"),
                             start=True, stop=True)
            # f1a: c0 = gray, c1 = fx, c2 = fy  (anchor row ry=0, phase p=0)
            nc.vector.tensor_copy(f1a[:][:, 0, :], g1[:, 0, 0, :])
            STT(f1a[:][:, 1, 1:JL], d1[:, 0, 1:JL], 2.0,
                ps1[:][:, JL + 1:2 * JL], Alu.mult, Alu.add)
            TT(f1a[:][:, 1, 1:JL], f1a[:][:, 1, 1:JL], d1[:, 1, 1:JL],
               Alu.add)
            TT(f1a[:][:, 2, 1:JL], s1[:, 1, 1:JL], ps1[:][:, 1:JL],
               Alu.subtract)

            f1v = f1a[:][:, :, 2:2 + GXL]                   # [128, 3, 68]
            f1b4 = f1v.unsqueeze(1).broadcast_to([128, 4, 3, GXL])
            f1b7 = f1v.unsqueeze(1).broadcast_to([128, 7, 3, GXL])

            # ---------------- correlation: early phase (ov=0) -------------
            # slots 8-11 read feat2 directly, overlapping the f2px DMAs
            prodE = prod[:][:, 8:12, :, :]
            for dx in range(15):
                pw, j0 = (1 + dx) % 4, (1 + dx) // 4
                h = dx % 2
                cEh = cE[:][:, h]
                TT(prodE, f1b4, feat2[:][:, :, :, pw, j0:j0 + GXL], Alu.mult)
                TT(s2_t[:][:, 8:12, :], prod[:][:, 8:12, 0, :],
                   prod[:][:, 8:12, 1, :], Alu.add)
                TT(cEh, s2_t[:][:, 8:12, :], prod[:][:, 8:12, 2, :], Alu.add)
                TT(rowmax[:][:, 8:12, :], rowmax[:][:, 8:12, :], cEh, Alu.max)
                if h == 1:
                    # batched colmax tree over the dx-1, dx pair
                    TT(m8[:][:, :, 0:2, :], cE[:][:, :, 0:2, :],
                       cE[:][:, :, 2:4, :], Alu.max)
                    TT(colmax[:][:, dx - 1:dx + 1, :], m8[:][:, :, 0, :],
                       m8[:][:, :, 1, :], Alu.max)
                elif dx == 14:
                    TT(m8[:][:, 0, 0:2, :], cE[:][:, 0, 0:2, :],
                       cE[:][:, 0, 2:4, :], Alu.max)
                    TT(colmax[:][:, 14, :], m8[:][:, 0, 0, :],
                       m8[:][:, 0, 1, :], Alu.max)

            # ---------------- correlation: main phase ----------------
            prod7 = prod[:][:, 1:8, :, :]
            prod4 = prod[:][:, 12:16, :, :]
            for dx in range(15):
                pw, j0 = (1 + dx) % 4, (1 + dx) // 4
                h = dx % 2
                c = c16[:][:, h]
                TT(prod7, f1b7, f2px[:][:, 1:8, :, pw, j0:j0 + GXL], Alu.mult)
                TT(prod4, f1b4, f2px[:][:, 12:16, :, pw, j0:j0 + GXL],
                   Alu.mult)
                TT(s2_t[:][:, 1:8, :], prod[:][:, 1:8, 0, :],
                   prod[:][:, 1:8, 1, :], Alu.add)
                TT(c[:, 1:8, :], s2_t[:][:, 1:8, :], prod[:][:, 1:8, 2, :],
                   Alu.add)
                TT(s2_t[:][:, 12:16, :], prod[:][:, 12:16, 0, :],
                   prod[:][:, 12:16, 1, :], Alu.add)
                TT(c[:, 12:16, :], s2_t[:][:, 12:16, :],
                   prod[:][:, 12:16, 2, :], Alu.add)
                TT(rowmax[:][:, 1:8, :], rowmax[:][:, 1:8, :], c[:, 1:8, :],
                   Alu.max)
                TT(rowmax[:][:, 12:16, :], rowmax[:][:, 12:16, :],
                   c[:, 12:16, :], Alu.max)
                if h == 1:
                    # batched colmax tree over the dx-1, dx pair
                    # (slots 0 and 8-11 are NEG rails)
                    TT(m8[:], c16[:][:, :, 0:8, :], c16[:][:, :, 8:16, :],
                       Alu.max)
                    TT(m8[:][:, :, 0:4, :], m8[:][:, :, 0:4, :],
                       m8[:][:, :, 4:8, :], Alu.max)
                    TT(m8[:][:, :, 0:2, :], m8[:][:, :, 0:2, :],
                       m8[:][:, :, 2:4, :], Alu.max)
                    TT(m8[:][:, :, 0, :], m8[:][:, :, 0, :],
                       m8[:][:, :, 1, :], Alu.max)
                    TT(colmax[:][:, dx - 1:dx + 1, :],
                       colmax[:][:, dx - 1:dx + 1, :],
                       m8[:][:, :, 0, :], Alu.max)
                elif dx == 14:
                    TT(m8[:][:, 0], c16[:][:, 0, 0:8, :],
                       c16[:][:, 0, 8:16, :], Alu.max)
                    TT(m8[:][:, 0, 0:4, :], m8[:][:, 0, 0:4, :],
                       m8[:][:, 0, 4:8, :], Alu.max)
                    TT(m8[:][:, 0, 0:2, :], m8[:][:, 0, 0:2, :],
                       m8[:][:, 0, 2:4, :], Alu.max)
                    TT(m8[:][:, 0, 0, :], m8[:][:, 0, 0, :],
                       m8[:][:, 0, 1, :], Alu.max)
                    TT(colmax[:][:, 14, :], colmax[:][:, 14, :],
                       m8[:][:, 0, 0, :], Alu.max)

            # ---------------- argmax -> displacement grid ----------------
            def first_argmax(buf15, ch):
                # buf15: [128, 15, GXL] AP, slots = dy/dx index 0..14.
                # overlapping-slice max tree (slot 7 counted twice).
                nc.vector.tensor_tensor(m8[:][:, 0], buf15[:, 0:8, :],
                                        buf15[:, 7:15, :], Alu.max)
                TT(m8[:][:, 0, 0:4, :], m8[:][:, 0, 0:4, :],
                   m8[:][:, 0, 4:8, :], Alu.max)
                TT(m8[:][:, 0, 0:2, :], m8[:][:, 0, 0:2, :],
                   m8[:][:, 0, 2:4, :], Alu.max)
                TT(m_t[:], m8[:][:, 0, 0, :], m8[:][:, 0, 1, :], Alu.max)
                mb = m_t[:].unsqueeze(1).broadcast_to([128, 15, GXL])
                TT(wsum[:], buf15, mb, Alu.is_ge)
                TT(wsum[:], wsum[:], wslot, Alu.mult)
                TT(m8[:][:, 0], wsum[:][:, 0:8, :], wsum[:][:, 7:15, :],
                   Alu.max)
                TT(m8[:][:, 0, 0:4, :], m8[:][:, 0, 0:4, :],
                   m8[:][:, 0, 4:8, :], Alu.max)
                TT(m8[:][:, 0, 0:2, :], m8[:][:, 0, 0:2, :],
                   m8[:][:, 0, 2:4, :], Alu.max)
                TT(fm_t[:], m8[:][:, 0, 0, :], m8[:][:, 0, 1, :], Alu.max)
                # disp = (argmax-7)/512 = (8 - fm)/512 ; zero invalid anchors
                nc.vector.tensor_scalar(fm_t[:], fm_t[:], -1.0 / 512.0,
                                        8.0 / 512.0, Alu.mult, Alu.add)
                TT(grid[:][:, ch, :], fm_t[:], gmask, Alu.mult)

            first_argmax(rowmax[:][:, 1:16, :], 1)
            first_argmax(colmax[:], 0)

            # ---------------- smoothing H-pass (phase weights) -------------
            Wp = _phase_weights()
            hsc2 = hsc[:]
            for p in range(4):
                nc.vector.tensor_scalar_mul(
                    hsc2, grid[:][:, :, 0:64], float(Wp[p, 0]))
                for dd in range(1, 4):
                    STT(hsc2, grid[:][:, :, dd:dd + 64],
                        float(Wp[p, dd]), hsc2, Alu.mult, Alu.add)
                STT(hp[:][:, :, p:256:4], grid[:][:, :, 4:4 + 64],
                    float(Wp[p, 4]), hsc2, Alu.mult, Alu.add)

            # ---------------- V-pass (PE banded matmul) + normalize --------
            nc.vector.tensor_copy(bands2[:], bands)
            rhs = hp[:].rearrange("p c x -> p (c x)")
            for t in range(4):
                ps = psum_pool.tile([128, 512], F32, tag="vps")
                nc.tensor.matmul(ps[:], bands2[:][:, t, :], rhs,
                                 start=True, stop=True)
                v = vps[:][:, t]
                sqx, sqy = v[:, 0, :], v[:, 1, :]
                nq, nm = v[:, 2, :], v[:, 3, :]
                nq2, nr2 = v[:, 4, :], v[:, 5, :]
                nc.scalar.activation(sqx, ps[:][:, 0:256], Act.Square)
                nc.scalar.activation(sqy, ps[:][:, 256:512], Act.Square)
                # q = max(qx,1e-30)+qy ; mag = q * rsqrt(q)
                STT(nq, sqx, 1e-30, sqy, Alu.max, Alu.add)
                nc.scalar.activation(nm, nq, Act.Abs_reciprocal_sqrt)
                TT(nm, nm, nq, Alu.mult)
                # magc = max(mag,1e-6)+1e-6 ; 1/magc = ars(magc^2)
                nc.vector.tensor_scalar(nm, nm, 1e-6, 1e-6, Alu.max, Alu.add)
                nc.scalar.activation(nq2, nm, Act.Square)
                nc.scalar.activation(nr2, nq2, Act.Abs_reciprocal_sqrt)
                TT(outsb[:][:, t, 0, :], ps[:][:, 0:256], nr2, Alu.mult)
                TT(outsb[:][:, t, 1, :], ps[:][:, 256:512], nr2, Alu.mult)
                nc.sync.dma_start(out_d.ap()[:, t:t + 1],
                                  outsb[:][:, t:t + 1])

    nc.compile()
    return nc


_NC_CACHE = None


def _get_nc():
    global _NC_CACHE
    if _NC_CACHE is None:
        _NC_CACHE = build_program()
    return _NC_CACHE


def kernel(frame1, frame2):
    frame1 = np.asarray(frame1, dtype=np.float32)
    frame2 = np.asarray(frame2, dtype=np.float32)
    nc = _get_nc()
    in_maps = _host_inputs(frame1, frame2)
    res = run_bass_kernel_spmd(nc, in_maps, core_ids=list(range(8)))
    if res.exec_time_ns is not None:
        print(f"HW exec time: {res.exec_time_ns} ns")
    out = np.empty((B, 2, H, W), np.float32)
    for b in range(B):
        for w in range(2):
            o = res.results[2 * b + w]["out"]        # [128, 4, 2, 256]
            o = o.transpose(2, 1, 0, 3).reshape(2, H, 256)
            out[b, :, :, 256 * w:256 * w + 256] = o
    return out


# revision 56
# speedup vs baseline: 1.5057x; 1.1833x over previous
"""Dense optical flow kernel for Trainium2, 8-core SPMD.

Pipeline (per core = one (sample, x-half) pair), x-polyphase layout
(x = 4j + p) so every correlation window read is a dense stride-1 run:

  frames -> gray/sobel features (row-polyphase ry, col-polyphase p)
  -> l2-normalize f2 (ACT abs_reciprocal_sqrt + DVE Newton)
  -> replicated window tensor f2px -> 15x15 windowed correlation (f32)
    on DVE with dense mults/adds and dense pairwise max trees
  -> first-argmax -> displacement grid -> separable gaussian smoothing
    (phase H-pass on DVE, banded-matmul V-pass on PE)
  -> direction normalize (ACT-heavy, no Newton) -> full-res flow.

Cross-partition row shifts for the vertical sobel go through PE
shift-matmuls (off-diagonal identity) instead of SBUF->SBUF DMA.
"""

import numpy as np

import concourse.bacc as bacc
import concourse.tile as tile
from concourse import mybir
from concourse.ap import AP
from concourse.bass_utils import run_bass_kernel_spmd

F32 = mybir.dt.float32
Alu = mybir.AluOpType
Act = mybir.ActivationFunctionType
AX = mybir.AxisListType

H = 512
W = 512
B = 4
XL = 288          # per-core padded column span
JL = 72           # XL / 4 (x-polyphase)
GXL = 68          # local anchor columns (64 + 2 halo each side)
NEG = np.float32(-1.0e30)
POS = np.float32(3.0e38)

# consts layout offsets (fp32 elements per partition)
OFF_XMASK = 0                   # [4ry, 4p, 72j] = 1152
OFF_GMASK = 1152                # [68]
OFF_BANDS = 1220                # [4t, 128y] = 512
OFF_WS = 1732                   # [15, 68] = 1020 (natural dx order)
OFF_SHDN = 2752                 # [128]
OFF_SHUP = 2880                 # [128]
OFF_WSY = 3008                  # [15, 68] = 1020 (compact-rowmax dy order)
NCONST = 4028

# rowmax compact slot order -> dy index (0..14 ~ dy=-7..7):
# slots 0:7 = f2px slots 1..7 (dy 0..6), 7:11 = f2px 12..15 (dy 11..14),
# 11:15 = ov0 early slots (dy 7..10)
DYIDX = [0, 1, 2, 3, 4, 5, 6, 11, 12, 13, 14, 7, 8, 9, 10]


# ----------------------------------------------------------------------------
# constants (host side)
# ----------------------------------------------------------------------------

def _gaussian_sep():
    ax = np.arange(15) - 7
    g = np.exp(-(ax.astype(np.float64) ** 2) / (2.0 * 2.5 ** 2))
    return (g / g.sum())


def _phase_weights():
    g = _gaussian_sep()
    Wp = np.zeros((4, 5), np.float64)
    for p in range(4):
        for t in range(15):
            Wp[p, (p + t - 7) // 4 + 2] += g[t]
    return Wp.astype(np.float32)


def _band_matrices():
    # bands[t][v, y]: out_row(128t+y) = sum_v band[v, y] * hp[v]
    Wp = _phase_weights()
    bands = np.zeros((4, 128, 128), np.float32)
    for t in range(4):
        for y in range(128):
            yg = 128 * t + y
            v0, q = yg // 4, yg % 4
            for d in range(5):
                v = v0 + d - 2
                if 0 <= v < 128:
                    bands[t, v, y] = Wp[q, d]
    return bands


def _poly(a):
    """[..., 288] -> [..., 4p, 72j] with x = 4j + p."""
    return np.ascontiguousarray(
        a.reshape(*a.shape[:-1], JL, 4).swapaxes(-1, -2))


def _host_inputs(frame1, frame2):
    """Build the 8 per-core input maps."""
    bands = _band_matrices()
    # PE shift matrices: out[i] = sum_p lhsT[p, i] * in[p]
    shdn = np.zeros((128, 128), np.float32)   # out[i] = in[i-1]
    shup = np.zeros((128, 128), np.float32)   # out[i] = in[i+1]
    for i in range(1, 128):
        shdn[i - 1, i] = 1.0
        shup[i, i - 1] = 1.0
    in_maps = []
    for b in range(B):
        for w in range(2):
            xbase = 256 * w - 16
            sl1 = np.zeros((3, H, XL), np.float32)
            sl2 = np.zeros((3, H, XL), np.float32)
            lo, hi = max(0, xbase), min(W, xbase + XL)
            sl1[:, :, lo - xbase:hi - xbase] = frame1[b][:, :, lo:hi]
            sl2[:, :, lo - xbase:hi - xbase] = frame2[b][:, :, lo:hi]
            # [3c, 512, 288] -> [128v, 4ry, 3c, 4p, 72j]
            il1 = np.ascontiguousarray(
                _poly(sl1).reshape(3, 128, 4, 4, JL).transpose(1, 2, 0, 3, 4))
            il2 = np.ascontiguousarray(
                _poly(sl2).reshape(3, 128, 4, 4, JL).transpose(1, 2, 0, 3, 4))
            # column-validity mask for the gray plane, polyphase, ry-tiled
            xcols = xbase + np.arange(XL)
            valid = (xcols >= 0) & (xcols < W)
            xm = _poly(np.where(valid, POS, NEG).astype(np.float32))  # [4,72]
            xmask = np.tile(xm.reshape(-1), 4)                        # ry x 4
            # anchor-validity mask
            gxg = 64 * w - 2 + np.arange(GXL)
            gm = ((gxg >= 0) & (gxg < 128)).astype(np.float32)
            ws = np.repeat((15.0 - np.arange(15, dtype=np.float32)),
                           GXL)
            wsy = np.repeat(15.0 - np.array(DYIDX, dtype=np.float32),
                            GXL)
            row = np.concatenate([xmask, gm, np.zeros(512, np.float32),
                                  ws, np.zeros(256, np.float32), wsy])
            consts = np.tile(row[None, :], (128, 1))
            consts[:, OFF_BANDS:OFF_BANDS + 512] = \
                bands.transpose(1, 0, 2).reshape(128, 512)
            consts[:, OFF_SHDN:OFF_SHDN + 128] = shdn
            consts[:, OFF_SHUP:OFF_SHUP + 128] = shup
            in_maps.append({"f1s": il1.reshape(128, -1),
                            "f2s": il2.reshape(128, -1),
                            "consts": consts})
    return in_maps


# ----------------------------------------------------------------------------
# device program
# ----------------------------------------------------------------------------

def build_program():
    nc = bacc.Bacc("TRN2", target_bir_lowering=False, debug=False)

    f1s_d = nc.dram_tensor("f1s", [128, 4 * 3 * 4 * JL], F32,
                           kind="ExternalInput")
    f2s_d = nc.dram_tensor("f2s", [128, 4 * 3 * 4 * JL], F32,
                           kind="ExternalInput")
    consts_d = nc.dram_tensor("consts", [128, NCONST], F32,
                              kind="ExternalInput")
    out_d = nc.dram_tensor("out", [128, 4, 2, 256], F32,
                           kind="ExternalOutput")

    with tile.TileContext(nc) as tc:
        with tc.tile_pool(name="main", bufs=1) as pool, \
             tc.tile_pool(name="psum", bufs=4, space="PSUM") as psum_pool:

            raw1 = pool.tile([128, 4, 3, 4, JL], F32)
            raw2 = pool.tile([128, 4, 3, 4, JL], F32)
            feat2 = pool.tile([128, 4, 3, 4, JL], F32)
            gray1 = pool.tile([128, 4, 4, JL], F32)
            sd = pool.tile([128, 4, 2, 4, JL], F32)     # s=0, d=1
            sd1 = pool.tile([128, 4, 2, JL], F32)       # f1, phase 0 only
            q_t = pool.tile([128, 4, 4, JL], F32)
            r0_t = pool.tile([128, 4, 4, JL], F32)
            a_t = pool.tile([128, 4, 4, JL], F32)
            consts = pool.tile([128, NCONST], F32)
            f2px = pool.tile([128, 11, 3, 4, JL], F32)
            f1a = pool.tile([128, 3, JL], F32)
            # correlation scratch; aliases of tensors dead by corr time
            s2_t = pool.tile([128, 16, GXL], F32, tag="a_t")
            prodG = pool.tile([128, 4, 4, 3, GXL], F32, tag="raw2")
            pm0 = pool.tile([128, 4, 11, GXL], F32)
            pm1 = pool.tile([128, 4, 11, GXL], F32)
            cEg = pool.tile([128, 4, 4, GXL], F32)
            mG = pool.tile([128, 4, 2, GXL], F32)
            m8 = pool.tile([128, 2, 8, GXL], F32)
            rowmax = pool.tile([128, 16, GXL], F32, tag="q_t")
            colmax = pool.tile([128, 15, GXL], F32, tag="r0_t")
            colmaxE = pool.tile([128, 15, GXL], F32)
            wsum = pool.tile([128, 15, GXL], F32)
            m_t = pool.tile([128, GXL], F32)
            fm_t = pool.tile([128, GXL], F32)
            grid = pool.tile([128, 2, GXL], F32)
            hp = pool.tile([128, 2, 256], F32)
            hsc = pool.tile([128, 2, 256], F32)
            tscr = pool.tile([128, 64], F32)
            bands2 = pool.tile([128, 4, 128], F32)
            # V-pass scratch aliases f2px (dead after main corr loop)
            vps = pool.tile([128, 4, 8, 256], F32, tag="f2px")
            outsb = pool.tile([128, 4, 2, 256], F32, tag="raw1")

            TT = nc.vector.tensor_tensor
            STT = nc.vector.scalar_tensor_tensor
            _touch_n = [0]

            def touch(ap):
                k = _touch_n[0] = _touch_n[0] + 1
                nc.vector.tensor_copy(tscr[:][32:33, k % 64:k % 64 + 1], ap)

            # ---------------- input DMAs ----------------
            # raw2 arrives as 4 ry-plane pieces so gray can start on the
            # first plane while the rest stream in (input BW ~160GB/s)
            f2s_r = f2s_d.ap().rearrange("p (r k) -> p r k", r=4)
            f2s_rc = f2s_d.ap().rearrange("p (r c k) -> p r c k", r=4, c=3)
            nc.sync.dma_start(raw2[:][:, 0, 0], f2s_rc[:, 0, 0])
            nc.sync.dma_start(raw2[:][:, 0, 1:3], f2s_rc[:, 0, 1:3])
            for ry in range(1, 4):
                nc.sync.dma_start(raw2[:][:, ry], f2s_r[:, ry])
            for p0 in range(0, 128, 32):
                nc.sync.dma_start(
                    raw1[:].rearrange("p r c q j -> p (r c q j)")[p0:p0 + 32],
                    f1s_d.ap()[p0:p0 + 32])
            nc.sync.dma_start(consts[:], consts_d.ap())
            touch(consts[:][32:33, 0:1])
            # preload the abs_reciprocal_sqrt act table (covers Square and
            # Copy too) so no mid-pipeline ACT_TABLE_LOAD occurs
            nc.scalar.activation(tscr[:][0:1, 0:2], tscr[:][0:1, 2:4],
                                 Act.Abs_reciprocal_sqrt)
            xmask = consts[:][:, OFF_XMASK:OFF_XMASK + 1152].rearrange(
                "p (r q j) -> p r q j", r=4, q=4)
            gmask = consts[:][:, OFF_GMASK:OFF_GMASK + GXL]
            bands = consts[:][:, OFF_BANDS:OFF_BANDS + 512].rearrange(
                "p (t y) -> p t y", t=4)
            wslot = consts[:][:, OFF_WS:OFF_WS + 15 * GXL].rearrange(
                "p (s g) -> p s g", s=15)
            wsloty = consts[:][:, OFF_WSY:OFF_WSY + 15 * GXL].rearrange(
                "p (s g) -> p s g", s=15)
            shdn = consts[:][:, OFF_SHDN:OFF_SHDN + 128]
            shup = consts[:][:, OFF_SHUP:OFF_SHUP + 128]
            # stage the V-pass band matrices out of the critical path
            nc.vector.tensor_copy(bands2[:], bands)

            # f2px out-of-image corners (overwritten where valid by DMA);
            # gpsimd memsets overlap the input DMA wait
            nc.gpsimd.memset(f2px[:][0:2, 0:7, 0, :, :], float(NEG))
            nc.gpsimd.memset(f2px[:][0:2, 0:7, 1:3, :, :], 0.0)
            nc.gpsimd.memset(f2px[:][96:128, 7:11, 0, :, :], float(NEG))
            nc.gpsimd.memset(f2px[:][96:128, 7:11, 1:3, :, :], 0.0)

            # ---------------- frame2 features ----------------
            g2 = feat2[:][:, :, 0, :, :]                 # [128, 4ry, 4p, 72]
            r2v = raw2[:]
            # per-ry gray so each chunk starts when its input plane lands
            for ry in range(4):
                nc.vector.tensor_scalar_mul(g2[:, ry], r2v[:, ry, 0, :, :],
                                            0.299)
                STT(g2[:, ry], r2v[:, ry, 1, :, :], 0.587, g2[:, ry],
                    Alu.mult, Alu.add)
                STT(g2[:, ry], r2v[:, ry, 2, :, :], 0.114, g2[:, ry],
                    Alu.mult, Alu.add)
            # gray^2 on ACT, overlapped with the H-pass below
            nc.scalar.activation(q_t[:], g2, Act.Square)

            s_ = sd[:][:, :, 0, :, :]                    # [128, 4ry, 4p, 72]
            d_ = sd[:][:, :, 1, :, :]
            # d = g(x+1) - g(x-1); s = g(x-1) + 2 g(x) + g(x+1)
            TT(d_[:, :, 1:3, :], g2[:, :, 2:4, :], g2[:, :, 0:2, :],
               Alu.subtract)
            TT(d_[:, :, 0, 1:JL], g2[:, :, 1, 1:JL], g2[:, :, 3, 0:JL - 1],
               Alu.subtract)
            TT(d_[:, :, 3, 0:JL - 1], g2[:, :, 0, 1:JL], g2[:, :, 2, 0:JL - 1],
               Alu.subtract)
            STT(s_[:, :, 1:3, :], g2[:, :, 1:3, :], 2.0, g2[:, :, 0:2, :],
                Alu.mult, Alu.add)
            TT(s_[:, :, 1:3, :], s_[:, :, 1:3, :], g2[:, :, 2:4, :], Alu.add)
            STT(s_[:, :, 0, 1:JL], g2[:, :, 0, 1:JL], 2.0,
                g2[:, :, 3, 0:JL - 1], Alu.mult, Alu.add)
            TT(s_[:, :, 0, 1:JL], s_[:, :, 0, 1:JL], g2[:, :, 1, 1:JL],
               Alu.add)
            STT(s_[:, :, 3, 0:JL - 1], g2[:, :, 3, 0:JL - 1], 2.0,
                g2[:, :, 2, 0:JL - 1], Alu.mult, Alu.add)
            TT(s_[:, :, 3, 0:JL - 1], s_[:, :, 3, 0:JL - 1],
               g2[:, :, 0, 1:JL], Alu.add)
            # x boundary columns (x=0 and x=287)
            nc.vector.memset(sd[:][:, :, :, 0, 0:1], 0.0)
            nc.vector.memset(sd[:][:, :, :, 3, JL - 1:JL], 0.0)

            # cross-partition row shifts on PE: dn -> row 4v-1, up -> row 4v+4
            ps_sm1 = psum_pool.tile([128, 288], F32, tag="shift")
            ps_dm1 = psum_pool.tile([128, 288], F32, tag="shift")
            ps_sp1 = psum_pool.tile([128, 288], F32, tag="shift")
            ps_dp1 = psum_pool.tile([128, 288], F32, tag="shift")
            sd_f = sd[:].rearrange("p r s q j -> p r s (q j)")
            nc.tensor.matmul(ps_sm1[:], shdn, sd_f[:, 3, 0, :],
                             start=True, stop=True)
            nc.tensor.matmul(ps_dm1[:], shdn, sd_f[:, 3, 1, :],
                             start=True, stop=True)
            nc.tensor.matmul(ps_sp1[:], shup, sd_f[:, 0, 0, :],
                             start=True, stop=True)
            nc.tensor.matmul(ps_dp1[:], shup, sd_f[:, 0, 1, :],
                             start=True, stop=True)

            g1 = gray1[:]
            r1v = raw1[:]
            # only ry planes {0,1,3} feed the anchor-row features
            for sl in (slice(0, 2), slice(3, 4)):
                nc.vector.tensor_scalar_mul(g1[:, sl],
                                            r1v[:, sl, 0, :, :], 0.299)
                STT(g1[:, sl], r1v[:, sl, 1, :, :], 0.587, g1[:, sl],
                    Alu.mult, Alu.add)
                STT(g1[:, sl], r1v[:, sl, 2, :, :], 0.114, g1[:, sl],
                    Alu.mult, Alu.add)

            fx2 = feat2[:][:, :, 1, :, :]
            fy2 = feat2[:][:, :, 2, :, :]
            pq = lambda ps: ps[:].rearrange("p (q j) -> p q j", q=4)

            def vfx(ry, dm1, dp1):
                STT(fx2[:, ry, :, :], d_[:, ry, :, :], 2.0, dm1,
                    Alu.mult, Alu.add)
                TT(fx2[:, ry, :, :], fx2[:, ry, :, :], dp1, Alu.add)

            # fx planes first so the ACT square can start early
            vfx(1, d_[:, 0, :, :], d_[:, 2, :, :])
            vfx(2, d_[:, 1, :, :], d_[:, 3, :, :])
            vfx(0, pq(ps_dm1), d_[:, 1, :, :])
            vfx(3, d_[:, 2, :, :], pq(ps_dp1))
            HV = (slice(0, 2), slice(2, 4))
            for hf in HV:
                nc.scalar.activation(r0_t[:][:, hf], fx2[:, hf], Act.Square)
            TT(fy2[:, 1, :, :], s_[:, 2, :, :], s_[:, 0, :, :], Alu.subtract)
            TT(fy2[:, 0, :, :], s_[:, 1, :, :], pq(ps_sm1), Alu.subtract)
            nc.scalar.activation(a_t[:][:, 0:2], fy2[:, 0:2], Act.Square)
            TT(fy2[:, 2, :, :], s_[:, 3, :, :], s_[:, 1, :, :], Alu.subtract)
            TT(fy2[:, 3, :, :], pq(ps_sp1), s_[:, 2, :, :], Alu.subtract)
            nc.scalar.activation(a_t[:][:, 2:4], fy2[:, 2:4], Act.Square)

            # ---------------- frame2 normalize ----------------
            # processed in two ry-halves so ACT (square/rsqrt) overlaps the
            # DVE Newton/multiply chain of the other half
            for hf in HV:
                # q = max(fx2^2, 1e-24) + fy2^2 + gray^2
                STT(r0_t[:][:, hf], r0_t[:][:, hf], 1e-24, a_t[:][:, hf],
                    Alu.max, Alu.add)
                TT(q_t[:][:, hf], q_t[:][:, hf], r0_t[:][:, hf], Alu.add)
                nc.scalar.activation(r0_t[:][:, hf], q_t[:][:, hf],
                                     Act.Abs_reciprocal_sqrt)
            for hf in HV:
                # Newton: r1 = r0*(1.5 - 0.5*q*r0^2)
                TT(a_t[:][:, hf], r0_t[:][:, hf], r0_t[:][:, hf], Alu.mult)
                TT(a_t[:][:, hf], a_t[:][:, hf], q_t[:][:, hf], Alu.mult)
                nc.vector.tensor_scalar(a_t[:][:, hf], a_t[:][:, hf],
                                        -0.5, 1.5, Alu.mult, Alu.add)
                TT(r0_t[:][:, hf], r0_t[:][:, hf], a_t[:][:, hf], Alu.mult)
            for c in range(3):
                TT(feat2[:][:, :, c, :, :], feat2[:][:, :, c, :, :],
                   r0_t[:], Alu.mult)
            TT(g2, g2, xmask, Alu.min)


            # ---------------- f2px replication ----------------
            # ov=-2 (slots 1:4, slot 0 unused) and ov=-1 (slots 4:8) go over
            # DMA in 16-partition chunks spread across all queues; the ov=+1
            # group (slots 12:16) is built on the idle PE via shift-matmuls
            # with ACT copying PSUM->SBUF.
            f2px_f = f2px[:].rearrange("p s c q j -> p s (c q j)")
            feat2_f = feat2[:].rearrange("p r c q j -> p r (c q j)")
            for q0 in range(0, 128, 16):
                a, b = max(2, q0), q0 + 16
                if a < b:
                    nc.sync.dma_start(f2px_f[a:b, 0:3, :],
                                      feat2_f[a - 2:b - 2, 1:4, :])
            # ov=-1 (slots 3:7, shdn) and ov=+1 (slots 7:11, shup) via PE.
            # shdn copies all 128 partitions (row 0 gets the shift-matrix
            # zeros; gray is patched to NEG below); shup copies [0:127]
            # so partition 127 keeps its pre-set NEG/0 corner.
            for mat, dst0, pn in ((shdn, 3, 128), (shup, 7, 127)):
                for ry in range(4):
                    psa = psum_pool.tile([128, 512], F32, tag="vps")
                    psb = psum_pool.tile([128, 352], F32, tag="vps")
                    nc.tensor.matmul(psa[:], mat, feat2_f[:, ry, 0:512],
                                     start=True, stop=True)
                    nc.tensor.matmul(psb[:], mat, feat2_f[:, ry, 512:864],
                                     start=True, stop=True)
                    nc.scalar.copy(f2px_f[0:pn, dst0 + ry, 0:512],
                                   psa[:][0:pn])
                    nc.scalar.copy(f2px_f[0:pn, dst0 + ry, 512:864],
                                   psb[:][0:pn])
            nc.gpsimd.memset(f2px[:][0:1, 3:7, 0, :, :], float(NEG))

            # ---------------- frame1 features (anchors only) -------------
            s1 = sd1[:][:, :, 0, :]
            d1 = sd1[:][:, :, 1, :]
            # phase-0 H-pass only: d = g[p1,j] - g[p3,j-1]
            #                      s = g[p3,j-1] + 2 g[p0,j] + g[p1,j]
            TT(d1[:, :, 1:JL], g1[:, :, 1, 1:JL], g1[:, :, 3, 0:JL - 1],
               Alu.subtract)
            STT(s1[:, :, 1:JL], g1[:, :, 0, 1:JL], 2.0,
                g1[:, :, 3, 0:JL - 1], Alu.mult, Alu.add)
            TT(s1[:, :, 1:JL], s1[:, :, 1:JL], g1[:, :, 1, 1:JL], Alu.add)
            ps1 = psum_pool.tile([128, 2 * JL], F32, tag="shift")
            nc.tensor.matmul(ps1[:], shdn,
                             sd1[:][:, 3, :, :].rearrange("p s j -> p (s j)"),
                             start=True, stop=True)
            # f1a: c0 = gray, c1 = fx, c2 = fy  (anchor row ry=0, phase p=0)
            nc.vector.tensor_copy(f1a[:][:, 0, :], g1[:, 0, 0, :])
            STT(f1a[:][:, 1, 1:JL], d1[:, 0, 1:JL], 2.0,
                ps1[:][:, JL + 1:2 * JL], Alu.mult, Alu.add)
            TT(f1a[:][:, 1, 1:JL], f1a[:][:, 1, 1:JL], d1[:, 1, 1:JL],
               Alu.add)
            TT(f1a[:][:, 2, 1:JL], s1[:, 1, 1:JL], ps1[:][:, 1:JL],
               Alu.subtract)

            f1v = f1a[:][:, :, 2:2 + GXL]                   # [128, 3, 68]
            f1b4 = f1v.unsqueeze(1).broadcast_to([128, 4, 3, GXL])
            f1b7 = f1v.unsqueeze(1).broadcast_to([128, 7, 3, GXL])

            # ---------------- correlation: early phase (ov=0) -------------
            # slots 8-11 read feat2 directly, overlapping the f2px fill.
            # dx values sharing a window phase pw batch into one group:
            # their windows are overlapping stride-1 views (j0, j0+1, ...)
            s2g4 = s2_t[:].rearrange("p (d r) g -> p d r g", d=4)
            for gi, (pw, j0, ndx) in enumerate(
                    ((1, 0, 4), (2, 0, 4), (3, 0, 4), (0, 1, 3))):
                for c in range(3):
                    base = feat2[:][:, :, c, pw,
                                    j0:j0 + GXL].unsqueeze(1)
                    pr = list(base.ap)
                    win = AP(base.tensor, base.offset,
                             [pr[0], (1, ndx)] + pr[2:])
                    f1bc = f1a[:][:, c, 2:2 + GXL].unsqueeze(1).unsqueeze(
                        1).broadcast_to([128, ndx, 4, GXL])
                    TT(prodG[:][:, 0:ndx, :, c, :], f1bc, win, Alu.mult)
                TT(s2g4[:, 0:ndx], prodG[:][:, 0:ndx, :, 0, :],
                   prodG[:][:, 0:ndx, :, 1, :], Alu.add)
                TT(cEg[:][:, 0:ndx], s2g4[:, 0:ndx],
                   prodG[:][:, 0:ndx, :, 2, :], Alu.add)
                # rowmax: reduce over the dx group (overlap slice for n=3)
                TT(m8[:][:, :, 0:4, :], cEg[:][:, 0:2],
                   cEg[:][:, ndx - 2:ndx], Alu.max)
                TT(m8[:][:, 0, 4:8, :], m8[:][:, 0, 0:4, :],
                   m8[:][:, 1, 0:4, :], Alu.max)
                if gi == 0:
                    nc.vector.tensor_copy(rowmax[:][:, 11:15, :],
                                          m8[:][:, 0, 4:8, :])
                else:
                    TT(rowmax[:][:, 11:15, :], rowmax[:][:, 11:15, :],
                       m8[:][:, 0, 4:8, :], Alu.max)
                # colmaxE: reduce over ry per dx; slots gi::4 strided
                TT(mG[:][:, 0:ndx], cEg[:][:, 0:ndx, 0:2, :],
                   cEg[:][:, 0:ndx, 2:4, :], Alu.max)
                dx0 = (3 if pw == 0 else pw - 1)
                TT(colmaxE[:][:, dx0:dx0 + 4 * ndx - 3:4, :],
                   mG[:][:, 0:ndx, 0, :], mG[:][:, 0:ndx, 1, :], Alu.max)

            # ---------------- correlation: main phase ----------------
            # grouped by window phase pw like the early loop: per group the
            # three channel mults read overlapping stride-1 windows of all
            # 11 f2px slots, then adds/rowmax/colmax trees batch over the
            # whole dx group
            pm0v, pm1v = pm0[:], pm1[:]

            def mwin(c, pw, j0, ndx):
                base = f2px[:][:, :, c, pw, j0:j0 + GXL].unsqueeze(1)
                pr = list(base.ap)
                return AP(base.tensor, base.offset,
                          [pr[0], (1, ndx)] + pr[2:])

            for gi, (pw, j0, ndx) in enumerate(
                    ((1, 0, 4), (2, 0, 4), (3, 0, 4), (0, 1, 3))):
                dx0 = (3 if pw == 0 else pw - 1)
                for c, dst in ((0, pm0v), (1, pm1v)):
                    f1bc = f1a[:][:, c, 2:2 + GXL].unsqueeze(1).unsqueeze(
                        1).broadcast_to([128, ndx, 11, GXL])
                    TT(dst[:, 0:ndx], f1bc, mwin(c, pw, j0, ndx), Alu.mult)
                TT(pm0v[:, 0:ndx], pm0v[:, 0:ndx], pm1v[:, 0:ndx], Alu.add)
                f1bc = f1a[:][:, 2, 2:2 + GXL].unsqueeze(1).unsqueeze(
                    1).broadcast_to([128, ndx, 11, GXL])
                TT(pm1v[:, 0:ndx], f1bc, mwin(2, pw, j0, ndx), Alu.mult)
                TT(pm0v[:, 0:ndx], pm0v[:, 0:ndx], pm1v[:, 0:ndx], Alu.add)
                # rowmax: reduce over the dx group (pm1 rows are free now)
                TT(pm1v[:, 0:2], pm0v[:, 0:2], pm0v[:, ndx - 2:ndx],
                   Alu.max)
                TT(pm1v[:, 0], pm1v[:, 0], pm1v[:, 1], Alu.max)
                if gi == 0:
                    nc.vector.tensor_copy(rowmax[:][:, 0:11, :],
                                          pm1v[:, 0])
                else:
                    TT(rowmax[:][:, 0:11, :], rowmax[:][:, 0:11, :],
                       pm1v[:, 0], Alu.max)
                # colmax: reduce over the 11 slots per dx (overlap slices)
                TT(cEg[:][:, 0:ndx], pm0v[:, 0:ndx, 0:4, :],
                   pm0v[:, 0:ndx, 7:11, :], Alu.max)
                TT(cEg[:][:, 0:ndx], cEg[:][:, 0:ndx],
                   pm0v[:, 0:ndx, 3:7, :], Alu.max)
                TT(mG[:][:, 0:ndx], cEg[:][:, 0:ndx, 0:2, :],
                   cEg[:][:, 0:ndx, 2:4, :], Alu.max)
                TT(colmax[:][:, dx0:dx0 + 4 * ndx - 3:4, :],
                   mG[:][:, 0:ndx, 0, :], mG[:][:, 0:ndx, 1, :], Alu.max)

            # ---------------- argmax -> displacement grid ----------------
            # global max m_t is shared: max_dy rowmax == max_dx colmax
            TT(m8[:][:, 0], rowmax[:][:, 0:8, :], rowmax[:][:, 7:15, :],
               Alu.max)
            TT(m8[:][:, 0, 0:4, :], m8[:][:, 0, 0:4, :],
               m8[:][:, 0, 4:8, :], Alu.max)
            TT(m8[:][:, 0, 0:2, :], m8[:][:, 0, 0:2, :],
               m8[:][:, 0, 2:4, :], Alu.max)
            TT(m_t[:], m8[:][:, 0, 0, :], m8[:][:, 0, 1, :], Alu.max)
            mb = m_t[:].unsqueeze(1).broadcast_to([128, 15, GXL])

            def first_argmax(buf15, ch, wsl):
                # buf15: [128, 15, GXL] AP, slots = index 0..14; first
                # (smallest-index) argmax via is_ge * descending weights
                TT(wsum[:], buf15, mb, Alu.is_ge)
                TT(wsum[:], wsum[:], wsl, Alu.mult)
                TT(m8[:][:, 0], wsum[:][:, 0:8, :], wsum[:][:, 7:15, :],
                   Alu.max)
                TT(m8[:][:, 0, 0:4, :], m8[:][:, 0, 0:4, :],
                   m8[:][:, 0, 4:8, :], Alu.max)
                TT(m8[:][:, 0, 0:2, :], m8[:][:, 0, 0:2, :],
                   m8[:][:, 0, 2:4, :], Alu.max)
                TT(fm_t[:], m8[:][:, 0, 0, :], m8[:][:, 0, 1, :], Alu.max)
                # disp = (argmax-7)/512 = (8 - fm)/512 ; zero invalid anchors
                nc.vector.tensor_scalar(fm_t[:], fm_t[:], -1.0 / 512.0,
                                        8.0 / 512.0, Alu.mult, Alu.add)
                TT(grid[:][:, ch, :], fm_t[:], gmask, Alu.mult)

            first_argmax(rowmax[:][:, 0:15, :], 1, wsloty)
            TT(colmax[:], colmax[:], colmaxE[:], Alu.max)
            first_argmax(colmax[:], 0, wslot)

            # ---------------- smoothing H-pass (phase weights) -------------
            # 4 independent accumulation chains, emitted interleaved so
            # consecutive DVE instructions are independent
            Wp = _phase_weights()
            hscp = [hsc[:][:, :, 0:64], hsc[:][:, :, 64:128],
                    hsc[:][:, :, 128:192], hsc[:][:, :, 192:256]]
            for p in range(4):
                nc.vector.tensor_scalar_mul(
                    hscp[p], grid[:][:, :, 0:64], float(Wp[p, 0]))
            for dd in range(1, 4):
                for p in range(4):
                    STT(hscp[p], grid[:][:, :, dd:dd + 64],
                        float(Wp[p, dd]), hscp[p], Alu.mult, Alu.add)
            for p in range(4):
                STT(hp[:][:, :, p:256:4], grid[:][:, :, 4:4 + 64],
                    float(Wp[p, 4]), hscp[p], Alu.mult, Alu.add)

            # ---------------- V-pass (PE banded matmul) + normalize --------
            rhs = hp[:].rearrange("p c x -> p (c x)")
            for t in range(4):
                ps = psum_pool.tile([128, 512], F32, tag="vps")
                if t == 0:
                    nc.tensor.matmul(ps[:][:, 0:256], bands2[:][:, t, :],
                                     rhs[:, 0:256], start=True, stop=True)
                    nc.tensor.matmul(ps[:][:, 256:512], bands2[:][:, t, :],
                                     rhs[:, 256:512], start=True, stop=True)
                else:
                    nc.tensor.matmul(ps[:], bands2[:][:, t, :], rhs,
                                     start=True, stop=True)
                v = vps[:][:, t]
                sqx, sqy = v[:, 0, :], v[:, 1, :]
                nq, nm = v[:, 2, :], v[:, 3, :]
                nq2, nr2 = v[:, 4, :], v[:, 5, :]
                nc.scalar.activation(sqx, ps[:][:, 0:256], Act.Square)
                nc.scalar.activation(sqy, ps[:][:, 256:512], Act.Square)
                # q = max(qx,1e-30)+qy ; mag = q * rsqrt(q)
                STT(nq, sqx, 1e-30, sqy, Alu.max, Alu.add)
                nc.scalar.activation(nm, nq, Act.Abs_reciprocal_sqrt)
                TT(nm, nm, nq, Alu.mult)
                # magc = max(mag,1e-6)+1e-6 ; 1/magc = ars(magc^2)
                nc.vector.tensor_scalar(nm, nm, 1e-6, 1e-6, Alu.max, Alu.add)
                TT(nq2, nm, nm, Alu.mult)
                nc.scalar.activation(nr2, nq2, Act.Abs_reciprocal_sqrt)
                TT(outsb[:][:, t, 0, :], ps[:][:, 0:256], nr2, Alu.mult)
                TT(outsb[:][:, t, 1, :], ps[:][:, 256:512], nr2, Alu.mult)
                if t < 3:
                    nc.sync.dma_start(out_d.ap()[:, t:t + 1],
                                      outsb[:][:, t:t + 1])
                else:
                    # final block in two halves, issued from two sequencers
                    nc.sync.dma_start(out_d.ap()[0:64, t:t + 1],
                                      outsb[:][0:64, t:t + 1])
                    nc.scalar.dma_start(out_d.ap()[64:128, t:t + 1],
                                        outsb[:][64:128, t:t + 1])

    nc.compile()
    return nc


_NC_CACHE = None


def _get_nc():
    global _NC_CACHE
    if _NC_CACHE is None:
        _NC_CACHE = build_program()
    return _NC_CACHE


def kernel(frame1, frame2):
    frame1 = np.asarray(frame1, dtype=np.float32)
    frame2 = np.asarray(frame2, dtype=np.float32)
    nc = _get_nc()
    in_maps = _host_inputs(frame1, frame2)
    res = run_bass_kernel_spmd(nc, in_maps, core_ids=list(range(8)))
    if res.exec_time_ns is not None:
        print(f"HW exec time: {res.exec_time_ns} ns")
    out = np.empty((B, 2, H, W), np.float32)
    for b in range(B):
        for w in range(2):
            o = res.results[2 * b + w]["out"]        # [128, 4, 2, 256]
            o = o.transpose(2, 1, 0, 3).reshape(2, H, 256)
            out[b, :, :, 256 * w:256 * w + 256] = o
    return out
